# revision 1
# baseline (speedup 1.0000x reference)
"""CustomGCN (3-layer GCN + FF + skip + BN, eval mode) on 8 TRN2 NeuronCores.

Strategy (per sharding hint): nodes sharded across 8 cores (6250 rows each,
padded to 6272 = 49*128); edges partitioned by destination core/block; each
core owns the segment-sum for its node shard. Per layer the updated node
features are exchanged with an AllGather collective (bf16, node-major) so
every core can gather arbitrary source rows with indirect DMA.

The edge aggregation (including GCN symmetric norm and self loops) is
expressed as a sequence of small matmuls: for each destination block of 128
nodes, gather the source rows of its edges in chunks of 128 (one indirect
DMA per chunk, one row per partition) and multiply with a host-precomputed
weighted indicator matrix I[e, dst_local] = dinv[src]*dinv[dst], accumulating
in PSUM:  agg[feat, dst] += xg_chunk.T @ I_chunk.

Node-local compute (x@W matmuls, biases, relu/leaky-relu, BN affine) runs
feature-major ([128 feat x 6272 nodes] tiles) so per-feature parameters are
per-partition scalars.
"""

import os
import numpy as np
import ml_dtypes

N, D, E, L = 50000, 128, 500000, 3
EPS = 1e-5
SLOPE = 0.01
P = 8                      # cores
NS_RAW = N // P            # 6250
BLK = 128
NBLK = 49                  # ceil(6250/128)
NS = NBLK * BLK            # 6272 padded shard rows
NFULL = P * NS             # 50176
NSL = 512                  # node-matmul moving free dim
NSLICE = NS // NSL         # 12.25 -> handle remainder
_last_exec_ns = None


def _host_prep(x, edge_index):
    """Build per-core gather indices + weighted indicator tensors."""
    src = np.asarray(edge_index[0], dtype=np.int64)
    dst = np.asarray(edge_index[1], dtype=np.int64)
    deg = np.ones(N, np.float32)
    np.add.at(deg, dst, 1.0)
    dinv = (1.0 / np.sqrt(deg)).astype(np.float32)

    allsrc = np.concatenate([src, np.arange(N, dtype=np.int64)])
    alldst = np.concatenate([dst, np.arange(N, dtype=np.int64)])
    allw = np.concatenate([dinv[src] * dinv[dst], dinv * dinv]).astype(np.float32)

    core = alldst // NS_RAW
    dlc = alldst % NS_RAW
    block = dlc // BLK
    dl = dlc % BLK
    srcpos = (allsrc // NS_RAW) * NS + (allsrc % NS_RAW)

    key = core * NBLK + block
    order = np.argsort(key, kind="stable")
    key_s = key[order]
    counts = np.bincount(key, minlength=P * NBLK).reshape(P, NBLK)
    Kb = np.maximum(1, -(-counts.max(axis=0) // BLK))          # [NBLK] chunks per block
    coff = np.concatenate([[0], np.cumsum(Kb)])                # chunk offsets
    TC = int(Kb.sum())

    # rank of each edge within its (core, block) group
    gstart = np.concatenate([[0], np.cumsum(np.bincount(key_s, minlength=P * NBLK))])
    rank = np.arange(len(key_s)) - gstart[key_s]

    srcpos_s = srcpos[order]
    dl_s = dl[order]
    w_s = allw[order]
    core_s = key_s // NBLK
    block_s = key_s % NBLK

    j = coff[block_s] + rank // BLK       # chunk column within this core's tensors
    p = rank % BLK                        # partition

    idxs = np.zeros((P, BLK, TC), np.int32)
    inds = np.zeros((P, BLK, TC * BLK), np.float32)
    idxs[core_s, p, j] = srcpos_s
    inds[core_s, p, j * BLK + dl_s] = w_s
    return idxs, inds.astype(ml_dtypes.bfloat16), Kb, coff, TC


def _build_program(Kb, coff, TC):
    import concourse.bass as bass
    import concourse.bacc as bacc
    import concourse.mybir as mybir
    import concourse.tile as tile
    from concourse.masks import make_identity

    f32 = mybir.dt.float32
    bf16 = mybir.dt.bfloat16

    nc = bacc.Bacc("TRN2", target_bir_lowering=False, debug=False, num_devices=P)
    x0T_in = nc.declare_dram_parameter("x0T", [D, NS], f32, isOutput=False)
    x0bf_in = nc.declare_dram_parameter("x0bf", [NFULL, D], bf16, isOutput=False)
    idx_in = nc.declare_dram_parameter("idx", [BLK, TC], mybir.dt.int32, isOutput=False)
    ind_in = nc.declare_dram_parameter("ind", [BLK, TC * BLK], bf16, isOutput=False)
    wc_in = nc.declare_dram_parameter("wc", [L, D, D], f32, isOutput=False)
    wf_in = nc.declare_dram_parameter("wf", [L, D, D], f32, isOutput=False)
    wsk_in = nc.declare_dram_parameter("wsk", [L - 1, D, D], f32, isOutput=False)
    # vec columns: bc(0..2), bf(3..5), bsk(6..7), sBN(8..10), bBN(11..13)
    vec_in = nc.declare_dram_parameter("vec", [D, 14], f32, isOutput=False)
    y_out = nc.declare_dram_parameter("y", [D, NS], f32, isOutput=True)

    agsplit = os.environ.get("GCN_AGSPLIT", "1") == "1"
    if agsplit:
        H = D // 2
        agin = [[nc.dram_tensor(f"agin{i}_{h}", [NS, H], bf16) for h in range(2)]
                for i in range(L - 1)]
        agout = [[nc.dram_tensor(f"agout{i}_{h}", [NFULL, H], bf16,
                                 addr_space="Shared") for h in range(2)]
                 for i in range(L - 1)]
    else:
        agin = [nc.dram_tensor(f"agin{i}", [NS, D], bf16) for i in range(L - 1)]
        agout = [
            nc.dram_tensor(f"agout{i}", [NFULL, D], bf16, addr_space="Shared")
            for i in range(L - 1)
        ]

    KMAX = int(max(Kb))

    with tile.TileContext(nc) as tc:
        with (
            tc.tile_pool(name="const", bufs=1) as cpool,
            tc.tile_pool(name="big", bufs=1) as bigpool,
            tc.tile_pool(name="stream", bufs=3) as spool,
            tc.tile_pool(name="gx", bufs=24) as gxpool,
            tc.tile_pool(name="psum_e", bufs=4, space="PSUM") as pse,
            tc.tile_pool(name="psum_n", bufs=4, space="PSUM") as psn,
        ):
            # ---- constant loads ----
            idx_sb = cpool.tile([BLK, TC], mybir.dt.int32, tag="idx")
            nc.sync.dma_start(idx_sb[:], idx_in[:])
            vec_sb = cpool.tile([D, 14], f32, tag="vec")
            nc.sync.dma_start(vec_sb[:], vec_in[:])
            wtiles = {}
            for nm, t, cnt in (("wc", wc_in, L), ("wf", wf_in, L), ("wsk", wsk_in, L - 1)):
                for i in range(cnt):
                    w = cpool.tile([D, D], f32, tag=f"{nm}{i}")
                    nc.sync.dma_start(w[:], t[i])
                    wtiles[(nm, i)] = w
            ident = cpool.tile([D, D], f32, tag="ident")
            make_identity(nc, ident[:])

            # absorb idx-load wait into gpsimd before gathers
            scr_i = cpool.tile([1, 2], mybir.dt.int32, tag="scri")
            nc.gpsimd.tensor_copy(scr_i[0:1, 0:1], idx_sb[0:1, 0:1])

            # ---- state tiles (feature-major) ----
            # X: current features (and skip, always identical in this net)
            # A: aggregation target; T: temp; B0..B2: intermediates
            X = bigpool.tile([D, NS], f32, tag="x")
            nc.sync.dma_start(X[:], x0T_in[:])
            A = bigpool.tile([D, NS], f32, tag="agg")
            T = bigpool.tile([D, NS], f32, tag="tmp")
            B0 = bigpool.tile([D, NS], f32, tag="b0")
            B1 = bigpool.tile([D, NS], f32, tag="b1")
            B2 = bigpool.tile([D, NS], f32, tag="b2")

            scr_b = cpool.tile([1, 2], bf16, tag="scrb")

            for layer in range(L):
                if layer == 0:
                    gsrcs = [x0bf_in]
                elif agsplit:
                    gsrcs = agout[layer - 1]
                else:
                    gsrcs = [agout[layer - 1]]
                if layer > 0:
                    # absorb the collective wait(s) on gpsimd once per layer
                    for g in gsrcs:
                        nc.gpsimd.dma_start(scr_b[0:1, 0:2], g[0:1, 0:2])

                # ---- edge aggregation ----
                ablate = os.environ.get("GCN_ABLATE", "")
                for b in range(NBLK):
                    kb = int(Kb[b])
                    c0 = int(coff[b])
                    ind_t = spool.tile([BLK, KMAX * BLK], bf16, tag="ind")
                    nc.sync.dma_start(
                        ind_t[:, : kb * BLK],
                        ind_in[:, c0 * BLK:(c0 + kb) * BLK],
                    )
                    gts = []
                    for k in range(kb):
                        gt = gxpool.tile([BLK, D], bf16, tag="gx")
                        if ablate != "nogather":
                            if len(gsrcs) == 1:
                                nc.gpsimd.indirect_dma_start(
                                    out=gt[:],
                                    out_offset=None,
                                    in_=gsrcs[0][:],
                                    in_offset=bass.IndirectOffsetOnAxis(
                                        ap=idx_sb[:, c0 + k:c0 + k + 1], axis=0
                                    ),
                                )
                            else:
                                for h, g in enumerate(gsrcs):
                                    nc.gpsimd.indirect_dma_start(
                                        out=gt[:, h * (D // 2):(h + 1) * (D // 2)],
                                        out_offset=None,
                                        in_=g[:],
                                        in_offset=bass.IndirectOffsetOnAxis(
                                            ap=idx_sb[:, c0 + k:c0 + k + 1], axis=0
                                        ),
                                    )
                        gts.append(gt)
                    ps = pse.tile([D, BLK], f32, tag="pse")
                    if ablate == "noedge":
                        nc.vector.tensor_copy(A[:, b * BLK:(b + 1) * BLK],
                                              ind_t[:, :BLK])
                        continue
                    for k in range(kb):
                        nc.tensor.matmul(
                            ps[:],
                            lhsT=gts[k][:],
                            rhs=ind_t[:, k * BLK:(k + 1) * BLK],
                            start=(k == 0),
                            stop=(k == kb - 1),
                        )
                    nc.vector.tensor_copy(A[:, b * BLK:(b + 1) * BLK], ps[:])

                # ---- node phase ----
                # s1 = x_skip + bc[layer]  (x_skip == X); X is dead after this
                nc.vector.tensor_scalar_add(T[:], X[:], vec_sb[:, layer:layer + 1])
                for s in range(0, NS, NSL):
                    sl = slice(s, min(s + NSL, NS))
                    w = sl.stop - sl.start
                    pt = psn.tile([D, NSL], f32, tag="psn")
                    nc.tensor.matmul(pt[:, :w], lhsT=wtiles[("wc", layer)][:],
                                     rhs=A[:, sl], start=True, stop=True)
                    nc.vector.tensor_add(B0[:, sl], pt[:, :w], T[:, sl])
                nc.vector.tensor_scalar_max(B0[:], B0[:], 0.0)      # B0 = x1

                for s in range(0, NS, NSL):
                    sl = slice(s, min(s + NSL, NS))
                    w = sl.stop - sl.start
                    pt = psn.tile([D, NSL], f32, tag="psn")
                    nc.tensor.matmul(pt[:, :w], lhsT=wtiles[("wf", layer)][:],
                                     rhs=B0[:, sl], start=True, stop=True)
                    nc.scalar.activation(
                        B1[:, sl], pt[:, :w],
                        func=mybir.ActivationFunctionType.Lrelu,
                        bias=vec_sb[:, 3 + layer:4 + layer], scale=1.0, alpha=SLOPE,
                    )                                               # B1 = x2
                nc.vector.tensor_add(B2[:], B1[:], B0[:])
                nc.vector.tensor_scalar_max(B2[:], B2[:], 0.0)      # B2 = x3
                xs = B2
                xcur = B2
                if layer > 0:
                    for s in range(0, NS, NSL):
                        sl = slice(s, min(s + NSL, NS))
                        w = sl.stop - sl.start
                        pt = psn.tile([D, NSL], f32, tag="psn")
                        nc.tensor.matmul(pt[:, :w], lhsT=wtiles[("wsk", layer - 1)][:],
                                         rhs=B2[:, sl], start=True, stop=True)
                        nc.scalar.activation(
                            B1[:, sl], pt[:, :w],
                            func=mybir.ActivationFunctionType.Identity,
                            bias=vec_sb[:, 5 + layer:6 + layer], scale=1.0,
                        )                                           # B1 = sk
                    nc.vector.tensor_add(B0[:], B2[:], B1[:])
                    nc.vector.tensor_scalar_max(B0[:], B0[:], 0.0)  # B0 = x4
                    xs = B0
                    xcur = B0
                # BN affine:  T = xcur*sBN + bBN;  X = relu(T + xs)
                nc.vector.tensor_scalar(
                    T[:], xcur[:],
                    scalar1=vec_sb[:, 8 + layer:9 + layer],
                    scalar2=vec_sb[:, 11 + layer:12 + layer],
                    op0=mybir.AluOpType.mult, op1=mybir.AluOpType.add,
                )
                nc.vector.tensor_add(X[:], T[:], xs[:])
                nc.vector.tensor_scalar_max(X[:], X[:], 0.0)

                if layer < L - 1:
                    # cast + transpose shard to node-major bf16 and AllGather
                    for kblk in range(NBLK):
                        ptt = pse.tile([D, BLK], f32, tag="pse")
                        nc.tensor.transpose(
                            ptt[:], X[:, kblk * BLK:(kblk + 1) * BLK], ident[:]
                        )
                        xbT = spool.tile([BLK, D], bf16, tag="xbT")
                        nc.vector.tensor_copy(xbT[:], ptt[:])
                        if agsplit:
                            H = D // 2
                            for h in range(2):
                                nc.sync.dma_start(
                                    agin[layer][h][kblk * BLK:(kblk + 1) * BLK, :],
                                    xbT[:, h * H:(h + 1) * H],
                                )
                        else:
                            nc.sync.dma_start(
                                agin[layer][kblk * BLK:(kblk + 1) * BLK, :], xbT[:]
                            )
                    if os.environ.get("GCN_ABLATE", "") != "nocoll":
                        if agsplit:
                            for h in range(2):
                                nc.gpsimd.collective_compute(
                                    "AllGather",
                                    mybir.AluOpType.bypass,
                                    replica_groups=[list(range(P))],
                                    ins=[agin[layer][h][:]],
                                    outs=[agout[layer][h][:]],
                                )
                        else:
                            nc.gpsimd.collective_compute(
                                "AllGather",
                                mybir.AluOpType.bypass,
                                replica_groups=[list(range(P))],
                                ins=[agin[layer][:]],
                                outs=[agout[layer][:]],
                            )

            nc.sync.dma_start(y_out[:], X[:])
    nc.compile()
    return nc


def _run_pjrt(nc, in_maps, time_runs=0):
    """Run the compiled Bass program on the 8 cores via PJRT (axon), modeled
    on bass2jax.run_bass_via_pjrt but with optional repeat-timing (no output
    donation; all outputs are fully written by the kernel)."""
    import time as _time
    import jax
    import numpy as _np
    from jax.sharding import Mesh, PartitionSpec
    from jax.experimental.shard_map import shard_map
    import concourse.mybir as mybir
    from concourse import bass2jax
    from concourse.bass2jax import _bass_exec_p, partition_id_tensor

    bass2jax.install_neuronx_cc_hook()
    partition_name = nc.partition_id_tensor.name if nc.partition_id_tensor else None
    in_names, out_names, out_avals = [], [], []
    for alloc in nc.m.functions[0].allocations:
        if not isinstance(alloc, mybir.MemoryLocationSet):
            continue
        name = alloc.memorylocations[0].name
        if alloc.kind == "ExternalInput":
            if name != partition_name:
                in_names.append(name)
        elif alloc.kind == "ExternalOutput":
            out_names.append(name)
            out_avals.append(
                jax.core.ShapedArray(tuple(alloc.tensor_shape), mybir.dt.np(alloc.dtype))
            )
    n_params = len(in_names)
    zero_outs = [_np.zeros(a.shape, a.dtype) for a in out_avals]
    all_in_names = in_names + out_names + ([partition_name] if partition_name else [])

    def _body(*args):
        operands = list(args)
        if partition_name is not None:
            operands.append(partition_id_tensor())
        return tuple(_bass_exec_p.bind(
            *operands,
            out_avals=tuple(out_avals),
            in_names=tuple(all_in_names),
            out_names=tuple(out_names),
            lowering_input_output_aliases=(),
            sim_require_finite=True, sim_require_nnan=True, nc=nc,
        ))

    n_cores = len(in_maps)
    devices = jax.devices()[:n_cores]
    mesh = Mesh(_np.asarray(devices), ("core",))
    nspec = n_params + len(out_names)
    sharded = jax.jit(
        shard_map(_body, mesh=mesh,
                  in_specs=(PartitionSpec("core"),) * nspec,
                  out_specs=(PartitionSpec("core"),) * len(out_names),
                  check_rep=False),
        keep_unused=True,
    )
    concat_in = [
        _np.concatenate([_np.asarray(in_maps[c][nm]) for c in range(n_cores)], axis=0)
        for nm in in_names
    ] + [
        _np.zeros((n_cores * z.shape[0], *z.shape[1:]), z.dtype) for z in zero_outs
    ]
    dev_in = [jax.device_put(a) for a in concat_in]
    out_arrs = sharded(*dev_in)
    jax.block_until_ready(out_arrs)
    times = []
    for _ in range(time_runs):
        t0 = _time.perf_counter()
        o = sharded(*dev_in)
        jax.block_until_ready(o)
        times.append(_time.perf_counter() - t0)
    results = [
        {nm: _np.asarray(out_arrs[i]).reshape(n_cores, *out_avals[i].shape)[c]
         for i, nm in enumerate(out_names)}
        for c in range(n_cores)
    ]
    return results, (min(times) if times else None)


def kernel(**inputs):
    global _last_exec_ns

    x = np.asarray(inputs["x"], np.float32)
    edge_index = np.asarray(inputs["edge_index"])
    Wc = np.asarray(inputs["Wc"], np.float32)
    bc = np.asarray(inputs["bc"], np.float32)
    Wf = np.asarray(inputs["Wf"], np.float32)
    bf = np.asarray(inputs["bf"], np.float32)
    Wskip = np.asarray(inputs["Wskip"], np.float32)
    bskip = np.asarray(inputs["bskip"], np.float32)
    gamma = np.asarray(inputs["gamma"], np.float32)
    beta = np.asarray(inputs["beta"], np.float32)
    run_mean = np.asarray(inputs["run_mean"], np.float32)
    run_var = np.asarray(inputs["run_var"], np.float32)

    idxs, inds, Kb, coff, TC = _host_prep(x, edge_index)

    xpad = np.zeros((NFULL, D), np.float32)
    for c in range(P):
        xpad[c * NS:c * NS + NS_RAW] = x[c * NS_RAW:(c + 1) * NS_RAW]
    x0bf = xpad.astype(ml_dtypes.bfloat16)

    sBN = (gamma / np.sqrt(run_var + EPS)).astype(np.float32)   # [L, D]
    bBN = (beta - run_mean * sBN).astype(np.float32)
    vec = np.stack(
        [bc[0], bc[1], bc[2], bf[0], bf[1], bf[2], bskip[0], bskip[1],
         sBN[0], sBN[1], sBN[2], bBN[0], bBN[1], bBN[2]], axis=1
    ).astype(np.float32)  # [D, 14]

    nc = _build_program(Kb, coff, TC)

    in_maps = []
    for c in range(P):
        in_maps.append({
            "x0T": xpad[c * NS:(c + 1) * NS].T.copy(),
            "x0bf": x0bf,
            "idx": idxs[c],
            "ind": inds[c],
            "wc": Wc, "wf": Wf, "wsk": Wskip,
            "vec": vec,
        })

    time_runs = int(os.environ.get("GCN_TIME_RUNS", "0"))
    results, tmin = _run_pjrt(nc, in_maps, time_runs=time_runs)
    _last_exec_ns = None if tmin is None else int(tmin * 1e9)

    out = np.empty((N, D), np.float32)
    for c in range(P):
        yc = results[c]["y"]  # [D, NS]
        out[c * NS_RAW:(c + 1) * NS_RAW] = yc.T[:NS_RAW]
    return out



# revision 3
# speedup vs baseline: 1.0422x; 1.0422x over previous
"""CustomGCN (3-layer GCN + FF + skip + BN, eval mode) on 8 TRN2 NeuronCores.

Strategy (per sharding hint): nodes sharded across 8 cores (6250 rows each,
padded to 6272 = 49*128); edges partitioned by destination core/block; each
core owns the segment-sum for its node shard. Per layer the updated node
features are exchanged with an AllGather collective (bf16, node-major) so
every core can gather arbitrary source rows.

Edge aggregation: for each destination block of 128 nodes, the (weighted,
self-loop-augmented) segment-sum is a sequence of 128x128x128 matmuls
  agg[feat, dst] += gathered_chunk[edge, feat]^T-contract ind_chunk[edge, dst]
where ind_chunk is the weighted one-hot indicator built ON-CHIP by one DVE
tensor_scalar op per chunk: (iota[e,d] == dst[e]) * w[e].

Source-row gathering uses the batched gpsimd dma_gather custom instruction
(CounterMachine descriptor generation, ~0.34ns/row) instead of per-chunk
indirect DMAs (~430ns/row). dma_gather indexes are int16, so sources are
split in two halves (rows [0,32768) and [32768, 50176)) with per-half base
offsets; edges are grouped by (dst block, src half) on the host and padded
to 128-edge chunks (pad edges gather row 0 with weight 0).

Node-local compute (x@W matmuls, biases, relu/leaky-relu, BN affine) runs
feature-major ([128 feat x 6272 nodes] tiles) so per-feature parameters are
per-partition scalars.
"""

import os
import numpy as np
import ml_dtypes

N, D, E, L = 50000, 128, 500000, 3
EPS = 1e-5
SLOPE = 0.01
P = 8                      # cores
NS_RAW = N // P            # 6250
BLK = 128
NBLK = 49                  # ceil(6250/128)
NS = NBLK * BLK            # 6272 padded shard rows
NFULL = P * NS             # 50176
HALF = 32768               # int16 index limit for dma_gather
GB = 7                     # dst blocks per gather group
NG = NBLK // GB            # 7 gather groups
NSL = 512                  # node-matmul moving free dim
_last_exec_ns = None


def _host_prep(edge_index):
    """Partition edges by (core, dst block, src half); build per-core
    int16 gather indices (wrapped-16 layout) and per-chunk dst/weight
    scalar tensors for the on-chip indicator build."""
    src = np.asarray(edge_index[0], dtype=np.int64)
    dst = np.asarray(edge_index[1], dtype=np.int64)
    deg = np.ones(N, np.float32)
    np.add.at(deg, dst, 1.0)
    dinv = (1.0 / np.sqrt(deg)).astype(np.float32)

    allsrc = np.concatenate([src, np.arange(N, dtype=np.int64)])
    alldst = np.concatenate([dst, np.arange(N, dtype=np.int64)])
    allw = np.concatenate([dinv[src] * dinv[dst], dinv * dinv]).astype(np.float32)

    core = alldst // NS_RAW
    dlc = alldst % NS_RAW
    blk = dlc // BLK
    dl = dlc % BLK
    srcpos = (allsrc // NS_RAW) * NS + (allsrc % NS_RAW)
    half = (srcpos >= HALF).astype(np.int64)
    idxh = srcpos - half * HALF

    key = (core * NBLK + blk) * 2 + half
    order = np.argsort(key, kind="stable")
    key_s = key[order]
    counts = np.bincount(key_s, minlength=P * NBLK * 2).reshape(P, NBLK, 2)
    KS = (-(-counts // BLK)).max(axis=0)       # [NBLK, 2] chunks per (block, half)

    # static chunk layout: groups of GB blocks; per (g,h) one dma_gather region
    cst = np.zeros((NBLK, 2), np.int64)        # chunk offset of block within region
    nch = np.zeros((NG, 2), np.int64)          # chunks per (g,h) region
    for g in range(NG):
        for h in range(2):
            off = 0
            for b in range(g * GB, (g + 1) * GB):
                cst[b, h] = off
                off += KS[b, h]
            nch[g, h] = off
    gco = np.zeros((NG, 2), np.int64)          # global chunk offset of region
    t = 0
    for g in range(NG):
        for h in range(2):
            gco[g, h] = t
            t += nch[g, h]
    TCH = t

    # per-edge chunk/slot assignment
    gstart = np.concatenate([[0], np.cumsum(np.bincount(key_s, minlength=P * NBLK * 2))])
    rank = np.arange(len(key_s)) - gstart[key_s]
    core_s = key_s // (NBLK * 2)
    blk_s = (key_s // 2) % NBLK
    h_s = key_s % 2
    j = rank // BLK                            # chunk within (block, half)
    e = rank % BLK                             # slot (partition) within chunk
    g_s = blk_s // GB
    tglob = gco[g_s, h_s] + cst[blk_s, h_s] + j
    pos = (cst[blk_s, h_s] + j) * BLK + e      # gather position within region

    gidx16 = np.zeros((P, 16, TCH * 8), np.int16)
    dstv = np.zeros((P, BLK, TCH), np.float32)
    wv = np.zeros((P, BLK, TCH), np.float32)
    gidx16[core_s, pos % 16, gco[g_s, h_s] * 8 + pos // 16] = idxh[order]
    dstv[core_s, e, tglob] = dl[order]
    wv[core_s, e, tglob] = allw[order]
    gidx = np.tile(gidx16, (1, 8, 1))          # replicate across 8x16 partitions
    return gidx, dstv, wv, KS, nch, gco, cst, TCH


def _build_program(KS, nch, gco, cst, TCH):
    import concourse.bass as bass
    import concourse.bacc as bacc
    import concourse.mybir as mybir
    import concourse.tile as tile
    from concourse.masks import make_identity

    f32 = mybir.dt.float32
    bf16 = mybir.dt.bfloat16
    AF = mybir.ActivationFunctionType
    ALU = mybir.AluOpType

    nc = bacc.Bacc("TRN2", target_bir_lowering=False, debug=False, num_devices=P)
    x0T_in = nc.declare_dram_parameter("x0T", [D, NS], f32, isOutput=False)
    x0bf_in = nc.declare_dram_parameter("x0bf", [NFULL, D], bf16, isOutput=False)
    gidx_in = nc.declare_dram_parameter("gidx", [BLK, TCH * 8], mybir.dt.int16, isOutput=False)
    dstv_in = nc.declare_dram_parameter("dstv", [BLK, TCH], f32, isOutput=False)
    wv_in = nc.declare_dram_parameter("wv", [BLK, TCH], f32, isOutput=False)
    iota_in = nc.declare_dram_parameter("iota", [BLK, BLK], f32, isOutput=False)
    wc_in = nc.declare_dram_parameter("wc", [L, D, D], f32, isOutput=False)
    wf_in = nc.declare_dram_parameter("wf", [L, D, D], f32, isOutput=False)
    wsk_in = nc.declare_dram_parameter("wsk", [L - 1, D, D], f32, isOutput=False)
    # vec columns: bc(0..2), bf(3..5), bsk(6..7), sBN(8..10), bBN(11..13)
    vec_in = nc.declare_dram_parameter("vec", [D, 14], f32, isOutput=False)
    y_out = nc.declare_dram_parameter("y", [D, NS], f32, isOutput=True)

    agin = [nc.dram_tensor(f"agin{i}", [NS, D], bf16) for i in range(L - 1)]
    agout = [
        nc.dram_tensor(f"agout{i}", [NFULL, D], bf16, addr_space="Shared")
        for i in range(L - 1)
    ]

    NCH = [int(nch[:, h].max()) for h in range(2)]

    with tile.TileContext(nc) as tc:
        with (
            tc.tile_pool(name="const", bufs=1) as cpool,
            tc.tile_pool(name="big", bufs=1) as bigpool,
            tc.tile_pool(name="gbuf", bufs=2) as gpool,
            tc.tile_pool(name="ind", bufs=6) as ipool,
            tc.tile_pool(name="stream", bufs=3) as spool,
            tc.tile_pool(name="psum_e", bufs=4, space="PSUM") as pse,
            tc.tile_pool(name="psum_n", bufs=4, space="PSUM") as psn,
        ):
            # ---- constant loads ----
            gidx_sb = cpool.tile([BLK, TCH * 8], mybir.dt.int16, tag="gidx")
            nc.sync.dma_start(gidx_sb[:], gidx_in[:])
            dstv_sb = cpool.tile([BLK, TCH], f32, tag="dstv")
            nc.sync.dma_start(dstv_sb[:], dstv_in[:])
            wv_sb = cpool.tile([BLK, TCH], f32, tag="wv")
            nc.sync.dma_start(wv_sb[:], wv_in[:])
            iota_sb = cpool.tile([BLK, BLK], f32, tag="iota")
            nc.sync.dma_start(iota_sb[:], iota_in[:])
            vec_sb = cpool.tile([D, 14], f32, tag="vec")
            nc.sync.dma_start(vec_sb[:], vec_in[:])
            wtiles = {}
            for nm, tsr, cnt in (("wc", wc_in, L), ("wf", wf_in, L), ("wsk", wsk_in, L - 1)):
                for i in range(cnt):
                    w = cpool.tile([D, D], f32, tag=f"{nm}{i}")
                    nc.sync.dma_start(w[:], tsr[i])
                    wtiles[(nm, i)] = w
            ident = cpool.tile([D, D], f32, tag="ident")
            make_identity(nc, ident[:])

            # ---- state tiles (feature-major) ----
            X = bigpool.tile([D, NS], f32, tag="x")
            nc.sync.dma_start(X[:], x0T_in[:])
            A = bigpool.tile([D, NS], f32, tag="agg")
            B0 = bigpool.tile([D, NS], f32, tag="b0")
            B1 = bigpool.tile([D, NS], f32, tag="b1")
            B2 = bigpool.tile([D, NS], f32, tag="b2")

            scr_b = cpool.tile([1, 2], bf16, tag="scrb")

            for layer in range(L):
                src = x0bf_in if layer == 0 else agout[layer - 1]
                if layer > 0:
                    # absorb the collective wait on gpsimd once per layer
                    nc.gpsimd.dma_start(scr_b[0:1, 0:2], src[0:1, 0:2])

                # ---- edge aggregation ----
                # dma_gather is limited by the SWDGE descriptor ring
                # (~4096 descs) so each region is gathered in slabs of
                # SLABC chunks (single_packet=False: >64 descs/engine).
                SLABC = 14
                for g in range(NG):
                    gts = {}
                    for h in range(2):
                        n = int(nch[g, h])
                        if n == 0:
                            continue
                        gt = gpool.tile([BLK, NCH[h] * BLK], bf16, tag=f"g{h}")
                        rows = HALF if h == 0 else NFULL - HALF
                        for s0 in range(0, n, SLABC):
                            s1 = min(s0 + SLABC, n)
                            nc.gpsimd.dma_gather(
                                gt[:, s0 * BLK : s1 * BLK].rearrange("p (c e) -> p c e", e=BLK),
                                src[h * HALF : h * HALF + rows],
                                gidx_sb[:, (int(gco[g, h]) + s0) * 8 : (int(gco[g, h]) + s1) * 8],
                                (s1 - s0) * BLK,
                                (s1 - s0) * BLK,
                                D,
                                single_packet=False,
                            )
                        gts[h] = gt
                    for b in range(g * GB, (g + 1) * GB):
                        kb = int(KS[b, 0] + KS[b, 1])
                        ps = pse.tile([D, BLK], f32, tag="pse")
                        k = 0
                        for h in range(2):
                            for j in range(int(KS[b, h])):
                                t = int(gco[g, h] + cst[b, h]) + j
                                ind_t = ipool.tile([BLK, BLK], bf16, tag="ind")
                                nc.vector.tensor_scalar(
                                    ind_t[:], iota_sb[:],
                                    dstv_sb[:, t:t + 1], wv_sb[:, t:t + 1],
                                    op0=ALU.is_equal, op1=ALU.mult,
                                )
                                posn = int(cst[b, h]) + j
                                nc.tensor.matmul(
                                    ps[:],
                                    lhsT=gts[h][:, posn * BLK:(posn + 1) * BLK],
                                    rhs=ind_t[:],
                                    start=(k == 0),
                                    stop=(k == kb - 1),
                                )
                                k += 1
                        nc.scalar.activation(A[:, b * BLK:(b + 1) * BLK], ps[:], func=AF.Identity)

                # ---- node phase ----
                # B0 = relu(Wc@A + x_skip + bc)   (x_skip == X)
                for s in range(0, NS, NSL):
                    sl = slice(s, min(s + NSL, NS))
                    w = sl.stop - sl.start
                    pt = psn.tile([D, NSL], f32, tag="psn")
                    nc.tensor.matmul(pt[:, :w], lhsT=wtiles[("wc", layer)][:],
                                     rhs=A[:, sl], start=True, stop=True)
                    nc.vector.tensor_add(B0[:, sl], pt[:, :w], X[:, sl])
                nc.vector.tensor_scalar(B0[:], B0[:], vec_sb[:, layer:layer + 1], 0.0,
                                        op0=ALU.add, op1=ALU.max)       # B0 = x1
                # B1 = lrelu(Wf@B0 + bf)
                for s in range(0, NS, NSL):
                    sl = slice(s, min(s + NSL, NS))
                    w = sl.stop - sl.start
                    pt = psn.tile([D, NSL], f32, tag="psn")
                    nc.tensor.matmul(pt[:, :w], lhsT=wtiles[("wf", layer)][:],
                                     rhs=B0[:, sl], start=True, stop=True)
                    nc.scalar.activation(
                        B1[:, sl], pt[:, :w], func=AF.Lrelu,
                        bias=vec_sb[:, 3 + layer:4 + layer], scale=1.0, alpha=SLOPE,
                    )                                                   # B1 = x2
                nc.vector.tensor_add(B2[:], B1[:], B0[:])
                nc.vector.tensor_scalar_max(B2[:], B2[:], 0.0)          # B2 = x3
                xs = B2
                xcur = B2
                if layer > 0:
                    for s in range(0, NS, NSL):
                        sl = slice(s, min(s + NSL, NS))
                        w = sl.stop - sl.start
                        pt = psn.tile([D, NSL], f32, tag="psn")
                        nc.tensor.matmul(pt[:, :w], lhsT=wtiles[("wsk", layer - 1)][:],
                                         rhs=B2[:, sl], start=True, stop=True)
                        nc.scalar.activation(
                            B1[:, sl], pt[:, :w], func=AF.Identity,
                            bias=vec_sb[:, 5 + layer:6 + layer], scale=1.0,
                        )                                               # B1 = sk
                    nc.vector.tensor_add(B0[:], B2[:], B1[:])
                    nc.vector.tensor_scalar_max(B0[:], B0[:], 0.0)      # B0 = x4
                    xs = B0
                    xcur = B0
                # BN affine + skip:  X = relu(xcur*sBN + bBN + xs)
                nc.vector.tensor_scalar(
                    B1[:], xcur[:],
                    vec_sb[:, 8 + layer:9 + layer], vec_sb[:, 11 + layer:12 + layer],
                    op0=ALU.mult, op1=ALU.add,
                )
                nc.vector.tensor_add(X[:], B1[:], xs[:])
                nc.vector.tensor_scalar_max(X[:], X[:], 0.0)

                if layer < L - 1:
                    # cast + transpose shard to node-major bf16 and AllGather
                    for kblk in range(NBLK):
                        ptt = pse.tile([D, BLK], f32, tag="pse")
                        nc.tensor.transpose(
                            ptt[:], X[:, kblk * BLK:(kblk + 1) * BLK], ident[:]
                        )
                        xbT = spool.tile([BLK, D], bf16, tag="xbT")
                        nc.scalar.activation(xbT[:], ptt[:], func=AF.Identity)
                        nc.sync.dma_start(
                            agin[layer][kblk * BLK:(kblk + 1) * BLK, :], xbT[:]
                        )
                    nc.gpsimd.collective_compute(
                        "AllGather",
                        mybir.AluOpType.bypass,
                        replica_groups=[list(range(P))],
                        ins=[agin[layer][:]],
                        outs=[agout[layer][:]],
                    )

            nc.sync.dma_start(y_out[:], X[:])
    nc.compile()
    return nc


def _run_pjrt(nc, in_maps, time_runs=0):
    """Run the compiled Bass program on the 8 cores via PJRT (axon), modeled
    on bass2jax.run_bass_via_pjrt but with optional repeat-timing (no output
    donation; all outputs are fully written by the kernel)."""
    import time as _time
    import jax
    import numpy as _np
    from jax.sharding import Mesh, PartitionSpec
    from jax.experimental.shard_map import shard_map
    import concourse.mybir as mybir
    from concourse import bass2jax
    from concourse.bass2jax import _bass_exec_p, partition_id_tensor

    bass2jax.install_neuronx_cc_hook()
    partition_name = nc.partition_id_tensor.name if nc.partition_id_tensor else None
    in_names, out_names, out_avals = [], [], []
    for alloc in nc.m.functions[0].allocations:
        if not isinstance(alloc, mybir.MemoryLocationSet):
            continue
        name = alloc.memorylocations[0].name
        if alloc.kind == "ExternalInput":
            if name != partition_name:
                in_names.append(name)
        elif alloc.kind == "ExternalOutput":
            out_names.append(name)
            out_avals.append(
                jax.core.ShapedArray(tuple(alloc.tensor_shape), mybir.dt.np(alloc.dtype))
            )
    n_params = len(in_names)
    zero_outs = [_np.zeros(a.shape, a.dtype) for a in out_avals]
    all_in_names = in_names + out_names + ([partition_name] if partition_name else [])

    def _body(*args):
        operands = list(args)
        if partition_name is not None:
            operands.append(partition_id_tensor())
        return tuple(_bass_exec_p.bind(
            *operands,
            out_avals=tuple(out_avals),
            in_names=tuple(all_in_names),
            out_names=tuple(out_names),
            lowering_input_output_aliases=(),
            sim_require_finite=True, sim_require_nnan=True, nc=nc,
        ))

    n_cores = len(in_maps)
    devices = jax.devices()[:n_cores]
    mesh = Mesh(_np.asarray(devices), ("core",))
    nspec = n_params + len(out_names)
    sharded = jax.jit(
        shard_map(_body, mesh=mesh,
                  in_specs=(PartitionSpec("core"),) * nspec,
                  out_specs=(PartitionSpec("core"),) * len(out_names),
                  check_rep=False),
        keep_unused=True,
    )
    concat_in = [
        _np.concatenate([_np.asarray(in_maps[c][nm]) for c in range(n_cores)], axis=0)
        for nm in in_names
    ] + [
        _np.zeros((n_cores * z.shape[0], *z.shape[1:]), z.dtype) for z in zero_outs
    ]
    dev_in = [jax.device_put(a) for a in concat_in]
    out_arrs = sharded(*dev_in)
    jax.block_until_ready(out_arrs)
    times = []
    for _ in range(time_runs):
        t0 = _time.perf_counter()
        o = sharded(*dev_in)
        jax.block_until_ready(o)
        times.append(_time.perf_counter() - t0)
    results = [
        {nm: _np.asarray(out_arrs[i]).reshape(n_cores, *out_avals[i].shape)[c]
         for i, nm in enumerate(out_names)}
        for c in range(n_cores)
    ]
    return results, (min(times) if times else None)


def kernel(**inputs):
    global _last_exec_ns

    x = np.asarray(inputs["x"], np.float32)
    edge_index = np.asarray(inputs["edge_index"])
    Wc = np.asarray(inputs["Wc"], np.float32)
    bc = np.asarray(inputs["bc"], np.float32)
    Wf = np.asarray(inputs["Wf"], np.float32)
    bf = np.asarray(inputs["bf"], np.float32)
    Wskip = np.asarray(inputs["Wskip"], np.float32)
    bskip = np.asarray(inputs["bskip"], np.float32)
    gamma = np.asarray(inputs["gamma"], np.float32)
    beta = np.asarray(inputs["beta"], np.float32)
    run_mean = np.asarray(inputs["run_mean"], np.float32)
    run_var = np.asarray(inputs["run_var"], np.float32)

    gidx, dstv, wv, KS, nch, gco, cst, TCH = _host_prep(edge_index)

    xpad = np.zeros((NFULL, D), np.float32)
    for c in range(P):
        xpad[c * NS:c * NS + NS_RAW] = x[c * NS_RAW:(c + 1) * NS_RAW]
    x0bf = xpad.astype(ml_dtypes.bfloat16)

    sBN = (gamma / np.sqrt(run_var + EPS)).astype(np.float32)   # [L, D]
    bBN = (beta - run_mean * sBN).astype(np.float32)
    vec = np.stack(
        [bc[0], bc[1], bc[2], bf[0], bf[1], bf[2], bskip[0], bskip[1],
         sBN[0], sBN[1], sBN[2], bBN[0], bBN[1], bBN[2]], axis=1
    ).astype(np.float32)  # [D, 14]
    iota = np.tile(np.arange(BLK, dtype=np.float32), (BLK, 1))

    nc = _build_program(KS, nch, gco, cst, TCH)

    in_maps = []
    for c in range(P):
        in_maps.append({
            "x0T": xpad[c * NS:(c + 1) * NS].T.copy(),
            "x0bf": x0bf,
            "gidx": gidx[c],
            "dstv": dstv[c],
            "wv": wv[c],
            "iota": iota,
            "wc": Wc, "wf": Wf, "wsk": Wskip,
            "vec": vec,
        })

    time_runs = int(os.environ.get("GCN_TIME_RUNS", "0"))
    results, tmin = _run_pjrt(nc, in_maps, time_runs=time_runs)
    _last_exec_ns = None if tmin is None else int(tmin * 1e9)

    out = np.empty((N, D), np.float32)
    for c in range(P):
        yc = results[c]["y"]  # [D, NS]
        out[c * NS_RAW:(c + 1) * NS_RAW] = yc.T[:NS_RAW]
    return out


# revision 4
# speedup vs baseline: 1.1195x; 1.0743x over previous
"""CustomGCN (3-layer GCN + FF + skip + BN, eval mode) on 8 TRN2 NeuronCores.

Strategy (per sharding hint): nodes sharded across 8 cores (6250 rows each,
padded to 6272 = 49*128); edges partitioned by destination core/block; each
core owns the segment-sum for its node shard. Per layer the updated node
features are exchanged with an AllGather collective (bf16, node-major) so
every core can gather arbitrary source rows.

Edge aggregation: for each destination block of 128 nodes, the (weighted,
self-loop-augmented) segment-sum is a sequence of 128x128x128 matmuls
  agg[feat, dst] += gathered_chunk[edge, feat]^T-contract ind_chunk[edge, dst]
where ind_chunk is the weighted one-hot indicator built ON-CHIP by one DVE
tensor_scalar op per chunk: (iota[e,d] == dst[e]) * w[e].

Source-row gathering uses the batched gpsimd dma_gather custom instruction
(CounterMachine descriptor generation, ~0.34ns/row) instead of per-chunk
indirect DMAs (~430ns/row). dma_gather indexes are int16, so sources are
split in two halves (rows [0,32768) and [32768, 50176)) with per-half base
offsets; edges are grouped by (dst block, src half) on the host and padded
to 128-edge chunks (pad edges gather row 0 with weight 0).

Node-local compute (x@W matmuls, biases, relu/leaky-relu, BN affine) runs
feature-major ([128 feat x 6272 nodes] tiles) so per-feature parameters are
per-partition scalars.
"""

import os
import numpy as np
import ml_dtypes

N, D, E, L = 50000, 128, 500000, 3
EPS = 1e-5
SLOPE = 0.01
P = 8                      # cores
NS_RAW = N // P            # 6250
BLK = 128
NBLK = 49                  # ceil(6250/128)
NS = NBLK * BLK            # 6272 padded shard rows
NFULL = P * NS             # 50176
HALF = 32768               # int16 index limit for dma_gather
GB = 7                     # dst blocks per gather group
NG = NBLK // GB            # 7 gather groups
NSL = 512                  # node-matmul moving free dim
_last_exec_ns = None


def _host_prep(edge_index):
    """Partition edges by (core, dst block, src half); build per-core
    int16 gather indices (wrapped-16 layout) and per-chunk dst/weight
    scalar tensors for the on-chip indicator build."""
    src = np.asarray(edge_index[0], dtype=np.int64)
    dst = np.asarray(edge_index[1], dtype=np.int64)
    deg = np.ones(N, np.float32)
    np.add.at(deg, dst, 1.0)
    dinv = (1.0 / np.sqrt(deg)).astype(np.float32)

    allsrc = np.concatenate([src, np.arange(N, dtype=np.int64)])
    alldst = np.concatenate([dst, np.arange(N, dtype=np.int64)])
    allw = np.concatenate([dinv[src] * dinv[dst], dinv * dinv]).astype(np.float32)

    core = alldst // NS_RAW
    dlc = alldst % NS_RAW
    blk = dlc // BLK
    dl = dlc % BLK
    srcpos = (allsrc // NS_RAW) * NS + (allsrc % NS_RAW)
    half = (srcpos >= HALF).astype(np.int64)
    idxh = srcpos - half * HALF

    key = (core * NBLK + blk) * 2 + half
    order = np.argsort(key, kind="stable")
    key_s = key[order]
    counts = np.bincount(key_s, minlength=P * NBLK * 2).reshape(P, NBLK, 2)
    KS = (-(-counts // BLK)).max(axis=0)       # [NBLK, 2] chunks per (block, half)

    # static chunk layout: groups of GB blocks; per (g,h) one dma_gather region
    cst = np.zeros((NBLK, 2), np.int64)        # chunk offset of block within region
    nch = np.zeros((NG, 2), np.int64)          # chunks per (g,h) region
    for g in range(NG):
        for h in range(2):
            off = 0
            for b in range(g * GB, (g + 1) * GB):
                cst[b, h] = off
                off += KS[b, h]
            nch[g, h] = off
    gco = np.zeros((NG, 2), np.int64)          # global chunk offset of region
    t = 0
    for g in range(NG):
        for h in range(2):
            gco[g, h] = t
            t += nch[g, h]
    TCH = t

    # per-edge chunk/slot assignment
    gstart = np.concatenate([[0], np.cumsum(np.bincount(key_s, minlength=P * NBLK * 2))])
    rank = np.arange(len(key_s)) - gstart[key_s]
    core_s = key_s // (NBLK * 2)
    blk_s = (key_s // 2) % NBLK
    h_s = key_s % 2
    j = rank // BLK                            # chunk within (block, half)
    e = rank % BLK                             # slot (partition) within chunk
    g_s = blk_s // GB
    tglob = gco[g_s, h_s] + cst[blk_s, h_s] + j
    pos = (cst[blk_s, h_s] + j) * BLK + e      # gather position within region

    gidx16 = np.zeros((P, 16, TCH * 8), np.int16)
    dstv = np.zeros((P, BLK, TCH), np.float32)
    wv = np.zeros((P, BLK, TCH), np.float32)
    gidx16[core_s, pos % 16, gco[g_s, h_s] * 8 + pos // 16] = idxh[order]
    dstv[core_s, e, tglob] = dl[order]
    wv[core_s, e, tglob] = allw[order]
    gidx = np.tile(gidx16, (1, 8, 1))          # replicate across 8x16 partitions
    return gidx, dstv, wv, KS, nch, gco, cst, TCH


def _build_program(KS, nch, gco, cst, TCH):
    import concourse.bass as bass
    import concourse.bacc as bacc
    import concourse.mybir as mybir
    import concourse.tile as tile
    from concourse.masks import make_identity

    f32 = mybir.dt.float32
    bf16 = mybir.dt.bfloat16
    AF = mybir.ActivationFunctionType
    ALU = mybir.AluOpType

    nc = bacc.Bacc("TRN2", target_bir_lowering=False, debug=False, num_devices=P)
    x0T_in = nc.declare_dram_parameter("x0T", [D, NS], f32, isOutput=False)
    x0bf_in = nc.declare_dram_parameter("x0bf", [NFULL, D], bf16, isOutput=False)
    gidx_in = nc.declare_dram_parameter("gidx", [BLK, TCH * 8], mybir.dt.int16, isOutput=False)
    dstv_in = nc.declare_dram_parameter("dstv", [BLK, TCH], f32, isOutput=False)
    wv_in = nc.declare_dram_parameter("wv", [BLK, TCH], f32, isOutput=False)
    iota_in = nc.declare_dram_parameter("iota", [BLK, BLK], f32, isOutput=False)
    wc_in = nc.declare_dram_parameter("wc", [L, D, D], f32, isOutput=False)
    wf_in = nc.declare_dram_parameter("wf", [L, D, D], f32, isOutput=False)
    wsk_in = nc.declare_dram_parameter("wsk", [L - 1, D, D], f32, isOutput=False)
    # vec columns: bc(0..2), bf(3..5), bsk(6..7), sBN(8..10), bBN(11..13)
    vec_in = nc.declare_dram_parameter("vec", [D, 14], f32, isOutput=False)
    y_out = nc.declare_dram_parameter("y", [D, NS], f32, isOutput=True)

    agin = [nc.dram_tensor(f"agin{i}", [NS, D], bf16) for i in range(L - 1)]
    agout = [
        nc.dram_tensor(f"agout{i}", [NFULL, D], bf16, addr_space="Shared")
        for i in range(L - 1)
    ]

    NCH = [int(nch[:, h].max()) for h in range(2)]

    with tile.TileContext(nc) as tc:
        with (
            tc.tile_pool(name="const", bufs=1) as cpool,
            tc.tile_pool(name="big", bufs=1) as bigpool,
            tc.tile_pool(name="gbuf", bufs=2) as gpool,
            tc.tile_pool(name="ind", bufs=6) as ipool,
            tc.tile_pool(name="stream", bufs=3) as spool,
            tc.tile_pool(name="psum_e", bufs=4, space="PSUM") as pse,
            tc.tile_pool(name="psum_n", bufs=4, space="PSUM") as psn,
        ):
            # ---- constant loads ----
            gidx_sb = cpool.tile([BLK, TCH * 8], mybir.dt.int16, tag="gidx")
            nc.sync.dma_start(gidx_sb[:], gidx_in[:])
            dstv_sb = cpool.tile([BLK, TCH], f32, tag="dstv")
            nc.sync.dma_start(dstv_sb[:], dstv_in[:])
            wv_sb = cpool.tile([BLK, TCH], f32, tag="wv")
            nc.sync.dma_start(wv_sb[:], wv_in[:])
            iota_sb = cpool.tile([BLK, BLK], f32, tag="iota")
            nc.sync.dma_start(iota_sb[:], iota_in[:])
            vec_sb = cpool.tile([D, 14], f32, tag="vec")
            nc.sync.dma_start(vec_sb[:], vec_in[:])
            wtiles = {}
            for nm, tsr, cnt in (("wc", wc_in, L), ("wf", wf_in, L), ("wsk", wsk_in, L - 1)):
                for i in range(cnt):
                    w = cpool.tile([D, D], f32, tag=f"{nm}{i}")
                    nc.sync.dma_start(w[:], tsr[i])
                    wtiles[(nm, i)] = w
            ident = cpool.tile([D, D], f32, tag="ident")
            make_identity(nc, ident[:])

            # ---- state tiles (feature-major) ----
            X = bigpool.tile([D, NS], f32, tag="x")
            nc.sync.dma_start(X[:], x0T_in[:])
            A = bigpool.tile([D, NS], f32, tag="agg")
            B0 = bigpool.tile([D, NS], f32, tag="b0")
            B1 = bigpool.tile([D, NS], f32, tag="b1")
            B2 = bigpool.tile([D, NS], f32, tag="b2")

            scr_b = cpool.tile([1, 2], bf16, tag="scrb")

            for layer in range(L):
                src = x0bf_in if layer == 0 else agout[layer - 1]
                if layer > 0:
                    # absorb the collective wait on gpsimd once per layer
                    nc.gpsimd.dma_start(scr_b[0:1, 0:2], src[0:1, 0:2])

                # ---- edge aggregation ----
                # dma_gather is limited by the SWDGE descriptor ring
                # (~4096 descs) so each region is gathered in slabs of
                # SLABC chunks (single_packet=False: >64 descs/engine).
                SLABC = 14
                for g in range(NG):
                    gts = {}
                    for h in range(2):
                        n = int(nch[g, h])
                        if n == 0:
                            continue
                        gt = gpool.tile([BLK, NCH[h] * BLK], bf16, tag=f"g{h}")
                        rows = HALF if h == 0 else NFULL - HALF
                        for s0 in range(0, n, SLABC):
                            s1 = min(s0 + SLABC, n)
                            nc.gpsimd.dma_gather(
                                gt[:, s0 * BLK : s1 * BLK].rearrange("p (c e) -> p c e", e=BLK),
                                src[h * HALF : h * HALF + rows],
                                gidx_sb[:, (int(gco[g, h]) + s0) * 8 : (int(gco[g, h]) + s1) * 8],
                                (s1 - s0) * BLK,
                                (s1 - s0) * BLK,
                                D,
                                single_packet=False,
                            )
                        gts[h] = gt
                    for b in range(g * GB, (g + 1) * GB):
                        kb = int(KS[b, 0] + KS[b, 1])
                        ps = pse.tile([D, BLK], f32, tag="pse")
                        k = 0
                        for h in range(2):
                            for j in range(int(KS[b, h])):
                                t = int(gco[g, h] + cst[b, h]) + j
                                ind_t = ipool.tile([BLK, BLK], bf16, tag="ind")
                                nc.vector.tensor_scalar(
                                    ind_t[:], iota_sb[:],
                                    dstv_sb[:, t:t + 1], wv_sb[:, t:t + 1],
                                    op0=ALU.is_equal, op1=ALU.mult,
                                )
                                posn = int(cst[b, h]) + j
                                nc.tensor.matmul(
                                    ps[:],
                                    lhsT=gts[h][:, posn * BLK:(posn + 1) * BLK],
                                    rhs=ind_t[:],
                                    start=(k == 0),
                                    stop=(k == kb - 1),
                                )
                                k += 1
                        nc.scalar.activation(A[:, b * BLK:(b + 1) * BLK], ps[:], func=AF.Identity)

                # ---- node phase ----
                # B0 = relu(Wc@A + x_skip + bc)   (x_skip == X)
                for s in range(0, NS, NSL):
                    sl = slice(s, min(s + NSL, NS))
                    w = sl.stop - sl.start
                    pt = psn.tile([D, NSL], f32, tag="psn")
                    nc.tensor.matmul(pt[:, :w], lhsT=wtiles[("wc", layer)][:],
                                     rhs=A[:, sl], start=True, stop=True)
                    nc.vector.tensor_add(B0[:, sl], pt[:, :w], X[:, sl])
                nc.vector.tensor_scalar(B0[:], B0[:], vec_sb[:, layer:layer + 1], 0.0,
                                        op0=ALU.add, op1=ALU.max)       # B0 = x1
                # B1 = lrelu(Wf@B0 + bf)
                for s in range(0, NS, NSL):
                    sl = slice(s, min(s + NSL, NS))
                    w = sl.stop - sl.start
                    pt = psn.tile([D, NSL], f32, tag="psn")
                    nc.tensor.matmul(pt[:, :w], lhsT=wtiles[("wf", layer)][:],
                                     rhs=B0[:, sl], start=True, stop=True)
                    nc.scalar.activation(
                        B1[:, sl], pt[:, :w], func=AF.Lrelu,
                        bias=vec_sb[:, 3 + layer:4 + layer], scale=1.0, alpha=SLOPE,
                    )                                                   # B1 = x2
                nc.vector.tensor_add(B2[:], B1[:], B0[:])
                nc.vector.tensor_scalar_max(B2[:], B2[:], 0.0)          # B2 = x3
                xs = B2
                xcur = B2
                if layer > 0:
                    for s in range(0, NS, NSL):
                        sl = slice(s, min(s + NSL, NS))
                        w = sl.stop - sl.start
                        pt = psn.tile([D, NSL], f32, tag="psn")
                        nc.tensor.matmul(pt[:, :w], lhsT=wtiles[("wsk", layer - 1)][:],
                                         rhs=B2[:, sl], start=True, stop=True)
                        nc.scalar.activation(
                            B1[:, sl], pt[:, :w], func=AF.Identity,
                            bias=vec_sb[:, 5 + layer:6 + layer], scale=1.0,
                        )                                               # B1 = sk
                    nc.vector.tensor_add(B0[:], B2[:], B1[:])
                    nc.vector.tensor_scalar_max(B0[:], B0[:], 0.0)      # B0 = x4
                    xs = B0
                    xcur = B0
                # BN affine + skip:  X = relu(xcur*sBN + bBN + xs)
                nc.vector.tensor_scalar(
                    B1[:], xcur[:],
                    vec_sb[:, 8 + layer:9 + layer], vec_sb[:, 11 + layer:12 + layer],
                    op0=ALU.mult, op1=ALU.add,
                )
                nc.vector.tensor_add(X[:], B1[:], xs[:])
                nc.vector.tensor_scalar_max(X[:], X[:], 0.0)

                if layer < L - 1:
                    # cast + transpose shard to node-major bf16 and AllGather
                    for kblk in range(NBLK):
                        ptt = pse.tile([D, BLK], f32, tag="pse")
                        nc.tensor.transpose(
                            ptt[:], X[:, kblk * BLK:(kblk + 1) * BLK], ident[:]
                        )
                        xbT = spool.tile([BLK, D], bf16, tag="xbT")
                        nc.scalar.activation(xbT[:], ptt[:], func=AF.Identity)
                        nc.sync.dma_start(
                            agin[layer][kblk * BLK:(kblk + 1) * BLK, :], xbT[:]
                        )
                    nc.gpsimd.collective_compute(
                        "AllGather",
                        mybir.AluOpType.bypass,
                        replica_groups=[list(range(P))],
                        ins=[agin[layer][:]],
                        outs=[agout[layer][:]],
                    )

            nc.sync.dma_start(y_out[:], X[:])
    nc.compile()
    return nc


def _run_pjrt(nc, in_maps, time_runs=0):
    """Run the compiled Bass program on the 8 cores via PJRT (axon), modeled
    on bass2jax.run_bass_via_pjrt but with optional repeat-timing (no output
    donation; all outputs are fully written by the kernel)."""
    import time as _time
    import jax
    import numpy as _np
    from jax.sharding import Mesh, PartitionSpec
    from jax.experimental.shard_map import shard_map
    import concourse.mybir as mybir
    from concourse import bass2jax
    from concourse.bass2jax import _bass_exec_p, partition_id_tensor

    bass2jax.install_neuronx_cc_hook()
    partition_name = nc.partition_id_tensor.name if nc.partition_id_tensor else None
    in_names, out_names, out_avals = [], [], []
    for alloc in nc.m.functions[0].allocations:
        if not isinstance(alloc, mybir.MemoryLocationSet):
            continue
        name = alloc.memorylocations[0].name
        if alloc.kind == "ExternalInput":
            if name != partition_name:
                in_names.append(name)
        elif alloc.kind == "ExternalOutput":
            out_names.append(name)
            out_avals.append(
                jax.core.ShapedArray(tuple(alloc.tensor_shape), mybir.dt.np(alloc.dtype))
            )
    n_params = len(in_names)
    zero_outs = [_np.zeros(a.shape, a.dtype) for a in out_avals]
    all_in_names = in_names + out_names + ([partition_name] if partition_name else [])

    def _body(*args):
        operands = list(args)
        if partition_name is not None:
            operands.append(partition_id_tensor())
        return tuple(_bass_exec_p.bind(
            *operands,
            out_avals=tuple(out_avals),
            in_names=tuple(all_in_names),
            out_names=tuple(out_names),
            lowering_input_output_aliases=(),
            sim_require_finite=True, sim_require_nnan=True, nc=nc,
        ))

    n_cores = len(in_maps)
    devices = jax.devices()[:n_cores]
    mesh = Mesh(_np.asarray(devices), ("core",))
    nspec = n_params + len(out_names)
    sharded = jax.jit(
        shard_map(_body, mesh=mesh,
                  in_specs=(PartitionSpec("core"),) * nspec,
                  out_specs=(PartitionSpec("core"),) * len(out_names),
                  check_rep=False),
        keep_unused=True,
    )
    concat_in = [
        _np.concatenate([_np.asarray(in_maps[c][nm]) for c in range(n_cores)], axis=0)
        for nm in in_names
    ] + [
        _np.zeros((n_cores * z.shape[0], *z.shape[1:]), z.dtype) for z in zero_outs
    ]
    dev_in = [jax.device_put(a) for a in concat_in]
    out_arrs = sharded(*dev_in)
    jax.block_until_ready(out_arrs)
    times = []
    for _ in range(time_runs):
        t0 = _time.perf_counter()
        o = sharded(*dev_in)
        jax.block_until_ready(o)
        times.append(_time.perf_counter() - t0)
    if os.environ.get("GCN_PROFILE", "0") == "1":
        import gauge.profiler
        with gauge.profiler.profile(
            kernel_dev_mode=True, profile_on_exit=False, bass_kernel=nc.m
        ) as prof:
            jax.block_until_ready(sharded(*dev_in))
        ntffs = prof.find_ntffs()
        print(f"GCN_PROFILE: ntffs={[(n.fname, n.device, n.execution) for n in ntffs]}")
        print(f"GCN_PROFILE: path={prof.profile_path.path}")
        try:
            res = prof.to_perfetto()
            print(f"GCN_PROFILE: perfetto={res}")
        except Exception as ex:
            print(f"GCN_PROFILE: to_perfetto failed: {ex}")
        for mi in sorted(prof._model_indices_with_json):
            try:
                print(f"GCN_PROFILE: total_time[{mi}]={prof.get_total_time(mi)}")
            except Exception as ex:
                print(f"GCN_PROFILE: total_time[{mi}] failed: {ex}")
    results = [
        {nm: _np.asarray(out_arrs[i]).reshape(n_cores, *out_avals[i].shape)[c]
         for i, nm in enumerate(out_names)}
        for c in range(n_cores)
    ]
    return results, (min(times) if times else None)


def kernel(**inputs):
    global _last_exec_ns

    x = np.asarray(inputs["x"], np.float32)
    edge_index = np.asarray(inputs["edge_index"])
    Wc = np.asarray(inputs["Wc"], np.float32)
    bc = np.asarray(inputs["bc"], np.float32)
    Wf = np.asarray(inputs["Wf"], np.float32)
    bf = np.asarray(inputs["bf"], np.float32)
    Wskip = np.asarray(inputs["Wskip"], np.float32)
    bskip = np.asarray(inputs["bskip"], np.float32)
    gamma = np.asarray(inputs["gamma"], np.float32)
    beta = np.asarray(inputs["beta"], np.float32)
    run_mean = np.asarray(inputs["run_mean"], np.float32)
    run_var = np.asarray(inputs["run_var"], np.float32)

    gidx, dstv, wv, KS, nch, gco, cst, TCH = _host_prep(edge_index)

    xpad = np.zeros((NFULL, D), np.float32)
    for c in range(P):
        xpad[c * NS:c * NS + NS_RAW] = x[c * NS_RAW:(c + 1) * NS_RAW]
    x0bf = xpad.astype(ml_dtypes.bfloat16)

    sBN = (gamma / np.sqrt(run_var + EPS)).astype(np.float32)   # [L, D]
    bBN = (beta - run_mean * sBN).astype(np.float32)
    vec = np.stack(
        [bc[0], bc[1], bc[2], bf[0], bf[1], bf[2], bskip[0], bskip[1],
         sBN[0], sBN[1], sBN[2], bBN[0], bBN[1], bBN[2]], axis=1
    ).astype(np.float32)  # [D, 14]
    iota = np.tile(np.arange(BLK, dtype=np.float32), (BLK, 1))

    nc = _build_program(KS, nch, gco, cst, TCH)

    in_maps = []
    for c in range(P):
        in_maps.append({
            "x0T": xpad[c * NS:(c + 1) * NS].T.copy(),
            "x0bf": x0bf,
            "gidx": gidx[c],
            "dstv": dstv[c],
            "wv": wv[c],
            "iota": iota,
            "wc": Wc, "wf": Wf, "wsk": Wskip,
            "vec": vec,
        })

    time_runs = int(os.environ.get("GCN_TIME_RUNS", "0"))
    results, tmin = _run_pjrt(nc, in_maps, time_runs=time_runs)
    _last_exec_ns = None if tmin is None else int(tmin * 1e9)

    out = np.empty((N, D), np.float32)
    for c in range(P):
        yc = results[c]["y"]  # [D, NS]
        out[c * NS_RAW:(c + 1) * NS_RAW] = yc.T[:NS_RAW]
    return out
